# revision 1
# baseline (speedup 1.0000x reference)
"""Local 7x7-window per-channel attention (SASA-style) on 8 TRN2 NeuronCores.

Reference computation per (batch, channel, pixel):
  q = groupconv1x1(x, wq);  k = groupconv1x1(pad(x), wk) + bk;  v = likewise wv/bv
  logits[k_off] = q * (k[p + k_off] + r_c[k_off])     (49 window offsets)
  out = sum_k softmax(logits)[k] * v[p + k_off]
where r_c[kh,kw] = rel_x[d,kh] for channel-in-group d<4, rel_y[d-4,kw] for d>=4.

Sharding: pure data-parallel. Core c owns image b=c//2, output-row half
h=c%2 (28 rows). On-core, the half is split into two 14-row quarters
stacked on SBUF partitions: partition p = quarter*64 + channel.
Each quarter's padded input slab is (64ch, 20rows, 62cols); no collectives.
"""

import sys

if "/opt/trn_rl_repo" not in sys.path:
    sys.path.insert(0, "/opt/trn_rl_repo")

import numpy as np

import concourse.bass as bass
import concourse.bacc as bacc
import concourse.tile as tile
from concourse import mybir
from concourse.bass_utils import run_bass_kernel_spmd

N_CORES = 8
KS = 7
PAD = 3
G = 8
DD = 8
C = 64  # channels
H = W = 56
B = 4
QR = 14          # output rows per quarter
PR = QR + 2 * PAD  # padded rows per quarter slab = 20
PW = W + 2 * PAD   # padded width = 62
CH = 7           # chunk rows for the attention loop
NCHUNK = QR // CH

F32 = mybir.dt.float32
BF16 = mybir.dt.bfloat16
ALU = mybir.AluOpType
ACTF = mybir.ActivationFunctionType


def _tree_fold(nc, T, nplanes):
    """Sum planes T[:, 0:nplanes] into T[:, 0] with in-place pairwise adds.

    T is a tile AP of shape (128, nplanes, R, C). Fat adds keep DVE in its
    2x bf16 mode; fp32 happens inside the ALU, rounding only at each store.
    """
    live = nplanes
    while live > 1:
        half = live // 2
        rem = live - 2 * half  # 0 or 1
        nc.vector.tensor_tensor(
            T[:, 0:half], T[:, 0:half], T[:, half:2 * half], ALU.add)
        if rem:
            if half >= 1:
                # fold the odd plane into plane 0 range next round
                nc.vector.tensor_tensor(
                    T[:, 0:1], T[:, 0:1], T[:, 2 * half:2 * half + 1], ALU.add)
        live = half


def build_nc():
    nc = bacc.Bacc("TRN2", target_bir_lowering=False, debug=False,
                   num_devices=N_CORES)
    x_ap = nc.dram_tensor("x", [128, PR, PW], F32, kind="ExternalInput").ap()
    wq_ap = nc.dram_tensor("wq", [128, 128], F32, kind="ExternalInput").ap()
    wk_ap = nc.dram_tensor("wk", [128, 128], F32, kind="ExternalInput").ap()
    wv_ap = nc.dram_tensor("wv", [128, 128], F32, kind="ExternalInput").ap()
    bk_ap = nc.dram_tensor("bk", [128, 1], F32, kind="ExternalInput").ap()
    bv_ap = nc.dram_tensor("bv", [128, 1], F32, kind="ExternalInput").ap()
    rt_ap = nc.dram_tensor("rt", [128, KS * KS], F32, kind="ExternalInput").ap()
    out_ap = nc.dram_tensor("out", [128, QR, W], F32, kind="ExternalOutput").ap()

    with tile.TileContext(nc) as tc:
        with tc.tile_pool(name="const", bufs=1) as constp, \
             tc.tile_pool(name="planes", bufs=1) as planesp, \
             tc.tile_pool(name="big", bufs=1) as bigp, \
             tc.tile_pool(name="small", bufs=2) as smallp, \
             tc.tile_pool(name="psum", bufs=2, space="PSUM") as psump:

            X = planesp.tile([128, PR, PW], F32)
            nc.sync.dma_start(out=X[:], in_=x_ap[:])
            Wq = constp.tile([128, 128], F32)
            nc.sync.dma_start(out=Wq[:], in_=wq_ap[:])
            Wk = constp.tile([128, 128], F32)
            nc.sync.dma_start(out=Wk[:], in_=wk_ap[:])
            Wv = constp.tile([128, 128], F32)
            nc.sync.dma_start(out=Wv[:], in_=wv_ap[:])
            BK = constp.tile([128, 1], F32)
            nc.sync.dma_start(out=BK[:], in_=bk_ap[:])
            BV = constp.tile([128, 1], F32)
            nc.sync.dma_start(out=BV[:], in_=bv_ap[:])
            RT = constp.tile([128, KS * KS], F32)
            nc.sync.dma_start(out=RT[:], in_=rt_ap[:])

            K = planesp.tile([128, PR, PW], F32)
            V = planesp.tile([128, PR, PW], BF16)
            Q = planesp.tile([128, QR, W], BF16)

            # k / v projections over the whole padded slab (1240 cols, 4x310)
            Xflat = X[:].rearrange("p h w -> p (h w)")
            Kflat = K[:].rearrange("p h w -> p (h w)")
            Vflat = V[:].rearrange("p h w -> p (h w)")
            ncol = PR * PW
            step = 310
            for (dst, wmat, bias) in ((Kflat, Wk, BK), (Vflat, Wv, BV)):
                for j in range(0, ncol, step):
                    ps = psump.tile([128, step], F32, tag="ps")
                    nc.tensor.matmul(ps[:], wmat[:], Xflat[:, j:j + step],
                                     start=True, stop=True)
                    nc.scalar.add(out=dst[:, j:j + step], in_=ps[:], add=bias[:])
            # q projection on the interior only (14x56, 2x392)
            for j in range(2):
                ps = psump.tile([128, CH * W], F32, tag="ps")
                nc.tensor.matmul(
                    ps[:], Wq[:],
                    X[:, PAD + j * CH: PAD + (j + 1) * CH, PAD:PAD + W],
                    start=True, stop=True)
                nc.scalar.copy(
                    out=Q[:, j * CH:(j + 1) * CH, :].rearrange("p h w -> p (h w)"),
                    in_=ps[:])

            # attention: single 14-row chunk, bf16 logits/weights.
            # Per kh-block pipeline keeps ACT (r-add + exp) and DVE
            # (q-mult, v-mult, folds) overlapped across blocks.
            L = bigp.tile([128, KS * KS, QR, W], BF16, tag="L")
            EV = bigp.tile([128, KS * KS, QR, W], BF16, tag="EV")
            qap = Q[:]
            qbcast = bass.AP(
                tensor=qap.tensor, offset=qap.offset,
                ap=[qap.ap[0], [0, KS], [W, QR], [1, W]])
            vap = V[:]

            def block_fold(T, b0):
                # planes b0..b0+6 summed into plane b0 (in place)
                nc.vector.tensor_tensor(
                    T[:, b0:b0 + 3], T[:, b0:b0 + 3], T[:, b0 + 3:b0 + 6],
                    ALU.add)
                for j in (1, 2, 6):
                    nc.vector.tensor_tensor(
                        T[:, b0:b0 + 1], T[:, b0:b0 + 1],
                        T[:, b0 + j:b0 + j + 1], ALU.add)

            # software pipeline: ACT order R0,R1,E0,R2,E1,...; DVE order
            # M0,M1,V0,F0,M2,V1,F1,... so neither engine stalls on the
            # other's in-order queue.
            def emit_radds(kh):
                b0 = kh * KS
                for kw in range(KS):
                    k = b0 + kw
                    nc.scalar.activation(
                        out=L[:, k], in_=K[:, kh:kh + QR, kw:kw + W],
                        func=ACTF.Identity, bias=RT[:, k:k + 1])

            def emit_qmult(kh):
                blk = L[:, kh * KS:(kh + 1) * KS]
                nc.vector.tensor_tensor(blk, blk, qbcast, ALU.mult)

            def emit_exp(kh):
                blk = L[:, kh * KS:(kh + 1) * KS]
                eblk = blk.rearrange("p k h w -> p (k h w)")
                nc.scalar.activation(out=eblk, in_=eblk, func=ACTF.Exp)

            def emit_tail(kh):
                b0 = kh * KS
                vwin = bass.AP(
                    tensor=vap.tensor, offset=vap.offset + kh * PW,
                    ap=[vap.ap[0], [1, KS], [PW, QR], [1, W]])
                nc.vector.tensor_tensor(
                    EV[:, b0:b0 + KS], L[:, b0:b0 + KS], vwin, ALU.mult)
                block_fold(EV, b0)
                block_fold(L, b0)

            for kh in range(KS):
                emit_radds(kh)
                emit_qmult(kh)
                if kh >= 1:
                    emit_exp(kh - 1)
                    emit_tail(kh - 1)
            emit_exp(KS - 1)
            emit_tail(KS - 1)

            # cross-block fold over planes {0, 7, ..., 42} (k-stride 7)
            def stride_planes(T, start, n):
                t = T[:]
                return bass.AP(
                    tensor=t.tensor,
                    offset=t.offset + start * KS * QR * W,
                    ap=[t.ap[0], [KS * QR * W, n], [W, QR], [1, W]])

            for T in (EV, L):
                nc.vector.tensor_tensor(
                    stride_planes(T, 0, 3), stride_planes(T, 0, 3),
                    stride_planes(T, 3, 3), ALU.add)
                for j in (1, 2, 6):
                    nc.vector.tensor_tensor(
                        stride_planes(T, 0, 1), stride_planes(T, 0, 1),
                        stride_planes(T, j, 1), ALU.add)
            P = QR * W
            Sf = L[:, 0].rearrange("p h w -> p (h w)")
            Of = EV[:, 0].rearrange("p h w -> p (h w)")
            # 1/S on ScalarE: exp(-ln S); Exp and Log share one table set
            LNS = smallp.tile([128, P], F32, tag="LNS")
            nc.scalar.activation(out=LNS[:], in_=Sf, func=ACTF.Ln)
            R = smallp.tile([128, P], F32, tag="R")
            nc.scalar.activation(out=R[:], in_=LNS[:], func=ACTF.Exp, scale=-1.0)
            OUTC = smallp.tile([128, P], F32, tag="OUTC")
            nc.vector.tensor_mul(OUTC[:], Of, R[:])
            nc.sync.dma_start(
                out=out_ap[:],
                in_=OUTC[:].rearrange("p (h w) -> p h w", h=QR))

    nc.compile()
    return nc


def shard_inputs(x, wq, wk, bk, wv, bv, rel_x, rel_y):
    """Full inputs -> list of 8 per-core input dicts (pure indexing/reshape)."""
    x_pad = np.zeros((B, C, H + 2 * PAD, W + 2 * PAD), dtype=np.float32)
    x_pad[:, :, PAD:PAD + H, PAD:PAD + W] = x

    def blockdiag(w):
        # lhsT layout: [cin, cout]; W64[g*8+i, g*8+o] = w[g, o, i]
        w64 = np.zeros((C, C), dtype=np.float32)
        for g in range(G):
            w64[g * DD:(g + 1) * DD, g * DD:(g + 1) * DD] = w[g].T
        w128 = np.zeros((128, 128), dtype=np.float32)
        w128[:64, :64] = w64
        w128[64:, 64:] = w64
        return w128

    wq128, wk128, wv128 = blockdiag(wq), blockdiag(wk), blockdiag(wv)
    bk128 = np.concatenate([bk, bk]).reshape(128, 1).astype(np.float32)
    bv128 = np.concatenate([bv, bv]).reshape(128, 1).astype(np.float32)

    rt64 = np.empty((C, KS, KS), dtype=np.float32)
    for g in range(G):
        for d in range(DD):
            if d < DD // 2:
                rt64[g * DD + d] = rel_x[d]          # (7,1) -> broadcast cols
            else:
                rt64[g * DD + d] = rel_y[d - DD // 2]  # (1,7) -> broadcast rows
    rt128 = np.concatenate([rt64, rt64]).reshape(128, KS * KS)
    rt128 = np.ascontiguousarray(rt128, dtype=np.float32)

    in_maps = []
    for core in range(N_CORES):
        b, half = divmod(core, 2)
        r0 = half * 2 * QR
        xs = np.empty((128, PR, PW), dtype=np.float32)
        xs[:64] = x_pad[b, :, r0:r0 + PR, :]
        xs[64:] = x_pad[b, :, r0 + QR:r0 + QR + PR, :]
        in_maps.append({
            "x": xs, "wq": wq128, "wk": wk128, "wv": wv128,
            "bk": bk128, "bv": bv128, "rt": rt128,
        })
    return in_maps


def unshard_output(results):
    out = np.empty((B, C, H, W), dtype=np.float32)
    for core in range(N_CORES):
        b, half = divmod(core, 2)
        r0 = half * 2 * QR
        r = results[core]["out"]  # (128, 14, 56)
        out[b, :, r0:r0 + QR, :] = r[:64]
        out[b, :, r0 + QR:r0 + 2 * QR, :] = r[64:]
    return out


_NC_CACHE = {}


def get_nc():
    if "nc" not in _NC_CACHE:
        _NC_CACHE["nc"] = build_nc()
    return _NC_CACHE["nc"]


def kernel(**inputs):
    nc = get_nc()
    in_maps = shard_inputs(**inputs)
    res = run_bass_kernel_spmd(nc, in_maps, core_ids=list(range(N_CORES)))
    return unshard_output(res.results)



# revision 2
# speedup vs baseline: 1.0023x; 1.0023x over previous
"""Local 7x7-window per-channel attention (SASA-style) on 8 TRN2 NeuronCores.

V3: three-engine balance (DVE / ACT / PE), no GpSimd compute (Pool work
steals DVE's shared SBUF port pair and halves DVE tensor_tensor throughput).

  - radd (L[k] = K_win + RT[k]): split ACT (Identity+bias ~0.94us/plane)
    / DVE (tensor_scalar 4x bf16 ~0.42us/plane).
  - qmult / evmult: DVE fat-7 tensor_tensor (~3.0us/block).
  - exp: ACT (~4.9us/block).
  - S-fold (sum_k E_k): PE identity-weight matmuls accumulating in PSUM
    f32 (2 x 392-col banks, 14 matmuls/block) - PE is otherwise idle.
  - EV-fold: DVE running-accumulator chain of fat-7 adds + tail.

Sharding: pure data-parallel. Core c owns image b=c//2, output-row half
h=c%2 (28 rows), split into two 14-row quarters stacked on partitions:
partition p = quarter*64 + channel. No collectives.
"""

import os
import sys

if "/opt/trn_rl_repo" not in sys.path:
    sys.path.insert(0, "/opt/trn_rl_repo")

import numpy as np

import concourse.bass as bass
import concourse.bacc as bacc
import concourse.tile as tile
from concourse import mybir
from concourse.bass_utils import run_bass_kernel_spmd

N_CORES = 8
KS = 7
PAD = 3
G = 8
DD = 8
C = 64
H = W = 56
B = 4
QR = 14
PR = QR + 2 * PAD   # 20
PW = W + 2 * PAD    # 62
P = QR * W          # 784
HP = P // 2         # 392

F32 = mybir.dt.float32
BF16 = mybir.dt.bfloat16
ALU = mybir.AluOpType
ACTF = mybir.ActivationFunctionType

# knobs
RADD_ASSIGN = os.environ.get("RADD_ASSIGN", "DDDDDDD" + "AAAADDD" * 6)
S_FOLD = os.environ.get("S_FOLD", "PE")     # "PE" or "D" (DVE chain)
EV_FOLD = os.environ.get("EV_FOLD", "D")    # "D" or "PE"
EVPE = [int(x) for x in os.environ.get("EVPE", "1,2,5").split(",") if x != ""]
WARMERS = int(os.environ.get("WARMERS", "4"))  # dummy PE matmuls per block
RECIP_DVE = os.environ.get("RECIP_DVE", "1") == "1"


def build_nc():
    nc = bacc.Bacc("TRN2", target_bir_lowering=False, debug=False,
                   num_devices=N_CORES)
    x_ap = nc.dram_tensor("x", [128, PR, PW], F32, kind="ExternalInput").ap()
    wq_ap = nc.dram_tensor("wq", [128, 128], F32, kind="ExternalInput").ap()
    wk_ap = nc.dram_tensor("wk", [128, 128], F32, kind="ExternalInput").ap()
    wv_ap = nc.dram_tensor("wv", [128, 128], F32, kind="ExternalInput").ap()
    cst_ap = nc.dram_tensor("cst", [128, 2 + KS * KS + 128], F32,
                            kind="ExternalInput").ap()
    out_ap = nc.dram_tensor("out", [128, QR, W], F32, kind="ExternalOutput").ap()

    with tile.TileContext(nc) as tc:
        with tc.tile_pool(name="const", bufs=1) as constp, \
             tc.tile_pool(name="planes", bufs=1) as planesp, \
             tc.tile_pool(name="big", bufs=1) as bigp, \
             tc.tile_pool(name="small", bufs=2) as smallp, \
             tc.tile_pool(name="psum", bufs=2, space="PSUM") as psump, \
             tc.tile_pool(name="psacc", bufs=1, space="PSUM") as psaccp:

            X = planesp.tile([128, PR, PW], F32)
            Wk = constp.tile([128, 128], F32)
            nc.scalar.dma_start(out=Wk[:], in_=wk_ap[:])
            # row bands matched to K-proj chunks so matmuls start early
            nc.sync.dma_start(out=X[:, 0:5], in_=x_ap[:, 0:5])
            nc.sync.dma_start(out=X[:, 5:10], in_=x_ap[:, 5:10])
            nc.sync.dma_start(out=X[:, 10:15], in_=x_ap[:, 10:15])
            nc.sync.dma_start(out=X[:, 15:20], in_=x_ap[:, 15:20])
            Wq = constp.tile([128, 128], F32)
            nc.scalar.dma_start(out=Wq[:], in_=wq_ap[:])
            Wv = constp.tile([128, 128], F32)
            nc.scalar.dma_start(out=Wv[:], in_=wv_ap[:])
            CST = constp.tile([128, 2 + KS * KS + 128], F32)
            nc.scalar.dma_start(out=CST[:], in_=cst_ap[:])
            BK = CST[:, 0:1]
            BV = CST[:, 1:2]
            RT = CST[:, 2:2 + KS * KS]
            I32 = CST[:, 2 + KS * KS:]
            IB = constp.tile([128, 128], BF16)
            nc.vector.tensor_copy(IB[:], I32)

            K = planesp.tile([128, PR, PW], BF16)
            V = planesp.tile([128, PR, PW], BF16)
            Q = planesp.tile([128, QR, W], BF16)

            Xflat = X[:].rearrange("p h w -> p (h w)")
            Kflat = K[:].rearrange("p h w -> p (h w)")
            Vflat = V[:].rearrange("p h w -> p (h w)")
            ncol = PR * PW
            step = 310

            def proj_chunk(dst, wmat, bias, j):
                ps = psump.tile([128, step], F32, tag="ps")
                nc.tensor.matmul(ps[:], wmat[:], Xflat[:, j:j + step],
                                 start=True, stop=True)
                if bias is None:
                    nc.scalar.copy(out=dst[:, j:j + step], in_=ps[:])
                else:
                    nc.scalar.add(out=dst[:, j:j + step], in_=ps[:],
                                  add=bias)

            # K rows 0..13 first so block-0 radds can start early
            for j in (0, 310, 620):
                proj_chunk(Kflat, Wk, BK, j)
            # Q (needed by qmult 0)
            for j in range(2):
                ps = psump.tile([128, KS * W], F32, tag="ps")
                nc.tensor.matmul(
                    ps[:], Wq[:],
                    X[:, PAD + j * KS: PAD + (j + 1) * KS, PAD:PAD + W],
                    start=True, stop=True)
                nc.scalar.copy(
                    out=Q[:, j * KS:(j + 1) * KS, :].rearrange(
                        "p h w -> p (h w)"),
                    in_=ps[:])
            proj_chunk(Kflat, Wk, BK, 930)

            L = bigp.tile([128, KS * KS, P], BF16, tag="L")
            EV = bigp.tile([128, KS * KS, P], BF16, tag="EV")

            qf = Q[:].rearrange("p h w -> p (h w)")
            qb = bass.AP(tensor=qf.tensor, offset=qf.offset,
                         ap=[qf.ap[0], [0, KS], [1, P]])
            vap = V[:]

            # PSUM accumulators for PE folds (each half = one bank)
            if S_FOLD == "PE":
                S0 = psaccp.tile([128, HP], F32, tag="S0")
                S1 = psaccp.tile([128, HP], F32, tag="S1")
            if EV_FOLD == "PE" or EVPE:
                E0 = psaccp.tile([128, HP], F32, tag="E0")
                E1 = psaccp.tile([128, HP], F32, tag="E1")
            if WARMERS:
                WRM = psaccp.tile([128, 64], F32, tag="WRM")

            def radd(k):
                kh, kw = divmod(k, KS)
                kwin = K[:, kh:kh + QR, kw:kw + W]
                if RADD_ASSIGN[k] == "A":
                    nc.scalar.activation(out=L[:, k], in_=kwin,
                                         func=ACTF.Identity,
                                         bias=RT[:, k:k + 1])
                else:
                    nc.vector.tensor_scalar(
                        out=L[:, k], in0=kwin, scalar1=RT[:, k:k + 1],
                        scalar2=None, op0=ALU.add)

            def qmult(b):
                blk = L[:, b * KS:(b + 1) * KS]
                nc.vector.tensor_tensor(blk, blk, qb, ALU.mult)

            def expblk(b):
                blk = L[:, b * KS:(b + 1) * KS].rearrange("p k x -> p (k x)")
                nc.scalar.activation(out=blk, in_=blk, func=ACTF.Exp)

            def evmult(b):
                vwin = bass.AP(
                    tensor=vap.tensor, offset=vap.offset + b * PW,
                    ap=[vap.ap[0], [1, KS], [PW, QR], [1, W]])
                eblk = L[:, b * KS:(b + 1) * KS].rearrange(
                    "p k (h w) -> p k h w", h=QR)
                oblk = EV[:, b * KS:(b + 1) * KS].rearrange(
                    "p k (h w) -> p k h w", h=QR)
                nc.vector.tensor_tensor(oblk, eblk, vwin, ALU.mult)

            def pe_fold_block(T, acc0, acc1, b):
                first = (b == 0)
                last = False
                for k in range(KS):
                    pl = T[:, b * KS + k]
                    nc.tensor.matmul(acc0[:], IB[:], pl[:, 0:HP],
                                     start=first and k == 0,
                                     stop=last and k == KS - 1)
                    nc.tensor.matmul(acc1[:], IB[:], pl[:, HP:P],
                                     start=first and k == 0,
                                     stop=last and k == KS - 1)

            def acc_chain(T, b):
                d = T[:, 0:KS]
                s = T[:, b * KS:(b + 1) * KS]
                nc.vector.tensor_tensor(d, d, s, ALU.add)

            def tail_fold(T):
                nc.vector.tensor_tensor(T[:, 0:3], T[:, 0:3], T[:, 3:6],
                                        ALU.add)
                nc.vector.tensor_tensor(T[:, 0:1], T[:, 0:1], T[:, 6:7],
                                        ALU.add)
                nc.vector.tensor_tensor(T[:, 0:1], T[:, 0:1], T[:, 1:2],
                                        ALU.add)
                nc.vector.tensor_tensor(T[:, 0:1], T[:, 0:1], T[:, 2:3],
                                        ALU.add)

            def pe_fold_partial(T, acc0, acc1, b, first, last):
                for k in range(KS):
                    pl = T[:, b * KS + k]
                    nc.tensor.matmul(acc0[:], IB[:], pl[:, 0:HP],
                                     start=first and k == 0,
                                     stop=last and k == KS - 1)
                    nc.tensor.matmul(acc1[:], IB[:], pl[:, HP:P],
                                     start=first and k == 0,
                                     stop=last and k == KS - 1)

            def warmer(k):
                # tiny matmul reading the just-written L plane: keeps the PE
                # HAM activity window non-idle so matmuls stay at 2.4 GHz
                nc.tensor.matmul(WRM[:], IB[:], L[:, k, 0:64],
                                 start=True, stop=True)

            # --- software pipeline over kh-blocks ---
            # ACT stream per slot: [radds-A(b+1), exp(b)] so the
            # radds(b+1) -> qmult(b+1) -> exp(b+1) chain overlaps exp(b);
            # DVE stream: [qm(b), radds-D(b+1), ev(b), evacc(b)].
            evpe_sorted = sorted(EVPE)

            def emit_radds(b):
                for kw in range(KS):
                    radd(b * KS + kw)
                    if WARMERS and kw in (0, 2, 4, 6)[:WARMERS]:
                        warmer(b * KS + kw)

            def tail_fold_at(T, base):
                b0 = base * KS
                nc.vector.tensor_tensor(T[:, b0:b0 + 3], T[:, b0:b0 + 3],
                                        T[:, b0 + 3:b0 + 6], ALU.add)
                nc.vector.tensor_tensor(T[:, b0:b0 + 1], T[:, b0:b0 + 1],
                                        T[:, b0 + 6:b0 + 7], ALU.add)
                nc.vector.tensor_tensor(T[:, b0:b0 + 1], T[:, b0:b0 + 1],
                                        T[:, b0 + 1:b0 + 2], ALU.add)
                nc.vector.tensor_tensor(T[:, b0:b0 + 1], T[:, b0:b0 + 1],
                                        T[:, b0 + 2:b0 + 3], ALU.add)

            chain_last = max(b for b in range(1, KS - 1) if b not in EVPE)

            def emit_ev_side(b):
                evmult(b)
                if b in EVPE:
                    pe_fold_partial(EV, E0, E1, b,
                                    first=(b == evpe_sorted[0]),
                                    last=(b == evpe_sorted[-1]))
                elif 1 <= b < KS - 1:
                    acc_chain(EV, b)
                    if b == chain_last:
                        # chain complete - fold its kw planes early
                        tail_fold_at(EV, 0)

            emit_radds(0)
            for b in range(KS):
                qmult(b)
                if b + 1 < KS:
                    emit_radds(b + 1)
                if b == 0:
                    for j in (0, 310, 620):
                        proj_chunk(Vflat, Wv, BV, j)
                elif b == 2:
                    proj_chunk(Vflat, Wv, BV, 930)
                if b < KS - 1:
                    expblk(b)
                    pe_fold_block(L, S0, S1, b)
                # ev side runs one slot delayed so DVE never waits on exp(b)
                if b >= 1:
                    emit_ev_side(b - 1)

            # --- endgame: block 6 in two pixel-column halves, pipelined ---
            from concourse.dve_ops import (RECIPROCAL_APPROX_FAST,
                                           RECIP_APPROX_FAST_CONSTS)
            cs = RECIP_APPROX_FAST_CONSTS
            R = smallp.tile([128, P], F32, tag="R")
            OSUM = smallp.tile([128, P], F32, tag="OSUM")
            OUTC = smallp.tile([128, P], F32, tag="OUTC")
            b6 = KS - 1
            b60 = b6 * KS
            Lt = L[:]
            EVt = EV[:]

            def half_ap(T, c0, ny):
                return bass.AP(
                    tensor=T.tensor, offset=T.offset + b60 * P + c0,
                    ap=[T.ap[0], [P, KS], [W, ny], [1, W]])

            def endgame_half(h):
                c0, c1 = h * HP, (h + 1) * HP
                Sh = (S0, S1)[h]
                Eh = (E0, E1)[h]
                ny = KS  # 7 rows per half
                # exp half (ACT)
                eb = bass.AP(tensor=Lt.tensor, offset=Lt.offset + b60 * P + c0,
                             ap=[Lt.ap[0], [P, KS], [1, HP]])
                nc.scalar.activation(out=eb, in_=eb, func=ACTF.Exp)
                # S-fold half (PE) closing the accumulation group
                for k in range(KS):
                    nc.tensor.matmul(Sh[:], IB[:], L[:, b60 + k, c0:c1],
                                     start=False, stop=(k == KS - 1))
                # ev half (DVE)
                vwin = bass.AP(
                    tensor=vap.tensor,
                    offset=vap.offset + b6 * PW + h * KS * PW,
                    ap=[vap.ap[0], [1, KS], [PW, ny], [1, W]])
                ebk = half_ap(Lt, c0, ny)
                obk = half_ap(EVt, c0, ny)
                nc.vector.tensor_tensor(obk, ebk, vwin, ALU.mult)
                # fold half: planes b60..b60+6 cols c0:c1 -> plane b60
                nc.vector.tensor_tensor(EV[:, b60:b60 + 3, c0:c1],
                                        EV[:, b60:b60 + 3, c0:c1],
                                        EV[:, b60 + 3:b60 + 6, c0:c1],
                                        ALU.add)
                nc.vector.tensor_tensor(EV[:, b60, c0:c1], EV[:, b60, c0:c1],
                                        EV[:, b60 + 6, c0:c1], ALU.add)
                nc.vector.tensor_tensor(EV[:, b60, c0:c1], EV[:, b60, c0:c1],
                                        EV[:, b60 + 1, c0:c1], ALU.add)
                nc.vector.tensor_tensor(EV[:, b60, c0:c1], EV[:, b60, c0:c1],
                                        EV[:, b60 + 2, c0:c1], ALU.add)
                # block6 + chain partial
                nc.vector.tensor_tensor(EV[:, b60, c0:c1], EV[:, b60, c0:c1],
                                        EV[:, 0, c0:c1], ALU.add)
                # recip half (DVE, after S stop)
                nc.vector._custom_dve(
                    RECIPROCAL_APPROX_FAST, out=R[:, c0:c1], in0=Sh[:],
                    s0=cs["s0"], s1=cs["s1"], imm2=cs["imm2"])
                # numerator: PE part + (chain + block6) part
                nc.vector.tensor_tensor(OSUM[:, c0:c1], Eh[:],
                                        EV[:, b60, c0:c1], ALU.add)
                nc.vector.tensor_mul(OUTC[:, c0:c1], OSUM[:, c0:c1],
                                     R[:, c0:c1])
                nc.sync.dma_start(
                    out=out_ap[:, h * KS:(h + 1) * KS],
                    in_=OUTC[:, c0:c1].rearrange("p (h w) -> p h w", h=KS))

            endgame_half(0)
            endgame_half(1)

    nc.compile()
    return nc


def shard_inputs(x, wq, wk, bk, wv, bv, rel_x, rel_y):
    x_pad = np.zeros((B, C, H + 2 * PAD, W + 2 * PAD), dtype=np.float32)
    x_pad[:, :, PAD:PAD + H, PAD:PAD + W] = x

    def blockdiag(w):
        w64 = np.zeros((C, C), dtype=np.float32)
        for g in range(G):
            w64[g * DD:(g + 1) * DD, g * DD:(g + 1) * DD] = w[g].T
        w128 = np.zeros((128, 128), dtype=np.float32)
        w128[:64, :64] = w64
        w128[64:, 64:] = w64
        return w128

    wq128, wk128, wv128 = blockdiag(wq), blockdiag(wk), blockdiag(wv)
    bk128 = np.concatenate([bk, bk]).reshape(128, 1).astype(np.float32)
    bv128 = np.concatenate([bv, bv]).reshape(128, 1).astype(np.float32)

    rt64 = np.empty((C, KS, KS), dtype=np.float32)
    for g in range(G):
        for d in range(DD):
            if d < DD // 2:
                rt64[g * DD + d] = rel_x[d]
            else:
                rt64[g * DD + d] = rel_y[d - DD // 2]
    rt128 = np.concatenate([rt64, rt64]).reshape(128, KS * KS)
    rt128 = np.ascontiguousarray(rt128, dtype=np.float32)
    ident = np.eye(128, dtype=np.float32)
    cst = np.concatenate([bk128, bv128, rt128, ident], axis=1)
    cst = np.ascontiguousarray(cst, dtype=np.float32)

    in_maps = []
    for core in range(N_CORES):
        b, half = divmod(core, 2)
        r0 = half * 2 * QR
        xs = np.empty((128, PR, PW), dtype=np.float32)
        xs[:64] = x_pad[b, :, r0:r0 + PR, :]
        xs[64:] = x_pad[b, :, r0 + QR:r0 + QR + PR, :]
        in_maps.append({
            "x": xs, "wq": wq128, "wk": wk128, "wv": wv128, "cst": cst,
        })
    return in_maps


def unshard_output(results):
    out = np.empty((B, C, H, W), dtype=np.float32)
    for core in range(N_CORES):
        b, half = divmod(core, 2)
        r0 = half * 2 * QR
        r = results[core]["out"]
        out[b, :, r0:r0 + QR, :] = r[:64]
        out[b, :, r0 + QR:r0 + 2 * QR, :] = r[64:]
    return out


_NC_CACHE = {}


def get_nc():
    if "nc" not in _NC_CACHE:
        _NC_CACHE["nc"] = build_nc()
    return _NC_CACHE["nc"]


def kernel(**inputs):
    nc = get_nc()
    in_maps = shard_inputs(**inputs)
    res = run_bass_kernel_spmd(nc, in_maps, core_ids=list(range(N_CORES)))
    return unshard_output(res.results)
